# revision 1
# baseline (speedup 1.0000x reference)
"""Trainium2 Bass kernel for the SE-attention block.

Math (per batch b):
    s[n]   = sum_c x[b,c,n]
    att[c] = sum_n x[b,c,n] * s[n]
    h      = relu(bn(W1 @ att))          (BN folded into scale/bias on host)
    a      = sigmoid(W2 @ h)
    out    = x[b] * a[:, None]

Sharding: data-parallel over batch B=16 across 8 cores (2 batches/core),
weights replicated, no collectives. The whole problem is latency/HBM
bound (~33.5 MB of DMA per core); sigmoid inputs are huge so the att
path must stay exact fp32 (bf16/fp32r flips near-threshold gates).

Per-core schedule (two batches, each split into four 1024-wide quarters
that pipeline through the engines):
  - SP HWDGE ring: all 32 quarter-loads up front (strict FIFO per ring,
    so nothing compute-dependent ever blocks a load).
  - DVE+POOL: tree-sum the 4 channel tiles per quarter (tA=x0+x1 on DVE,
    tB=x2+x3 and tA+tB on GpSimd).
  - PE:  sB = colsum(tree) broadcast to all 128 partitions in ONE
    matmul per 512-chunk using ones[128,128] weights into PSUM. fp32
    matmuls run LOW_HIGH (2 passes), so only the pre-reduced tree tile
    goes through the PE.
  - DVE: fused scalar_tensor_tensor: attq[128,1] = rowsum(x * sB).
  - PE:  tiny MLP matmuls (W1T/W2T pre-transposed on host); the W1T@att
    rank-1 accumulation is folded into the quarter stream.
  - ACT/DVE: BN+ReLU, sigmoid; out = x * a in place over each x quarter
    tile; stores balanced across the SP and ACT HWDGE rings (t<2 on SP,
    t>=2 on ACT) so the ~17 MB store tail drains on both rings at once.
"""

import numpy as np

try:
    import concourse.bass as bass
except ImportError:  # fresh grading dir: repo not on sys.path
    import sys

    for p in ("/opt/trn_rl_repo", "/root/.axon_site/_ro/trn_rl_repo"):
        if p not in sys.path:
            sys.path.insert(0, p)
    import concourse.bass as bass

import concourse.tile as tile
from concourse import bacc, mybir
from concourse.bass_utils import run_bass_kernel_spmd

F32 = mybir.dt.float32
AF = mybir.ActivationFunctionType
ALU = mybir.AluOpType

B, C, N = 16, 512, 4096
CR = 128          # squeeze dim C//4
NCORES = 8
BPC = B // NCORES  # batches per core
P = 128
CT = C // P        # channel tiles per batch
NH = N // 2        # psum half width (4 banks)
NCHUNK = 512       # matmul free-dim max (one psum bank)
BN_EPS = 1e-5

_nc_cache = None


def _build():
    nc = bacc.Bacc(None, target_bir_lowering=False)
    x = nc.declare_dram_parameter("x", [BPC, C, N], F32, isOutput=False)
    w1t = nc.declare_dram_parameter("w1t", [C, CR], F32, isOutput=False)
    w2t = nc.declare_dram_parameter("w2t", [CR, C], F32, isOutput=False)
    bns = nc.declare_dram_parameter("bns", [CR, 1], F32, isOutput=False)
    bnb = nc.declare_dram_parameter("bnb", [CR, 1], F32, isOutput=False)
    y = nc.declare_dram_parameter("y", [BPC, C, N], F32, isOutput=True)

    NQ = N // 4   # 1024-wide pipeline quarters
    QS = 4        # quarters per batch

    with tile.TileContext(nc) as tc:
        with (
            tc.tile_pool(name="consts", bufs=1) as consts,
            tc.tile_pool(name="x", bufs=2 * CT * QS) as xpool,
            tc.tile_pool(name="big", bufs=2) as big,
            tc.tile_pool(name="small", bufs=4 * CT) as small,
            tc.tile_pool(name="psum", bufs=2, space="PSUM") as psum,
            tc.tile_pool(name="dram", bufs=4, space="DRAM") as dram,
        ):
            ones128 = consts.tile([P, P], F32)
            nc.vector.memset(ones128, 1.0)
            w1t_sb = consts.tile([P, CT, CR], F32)
            nc.sync.dma_start(
                out=w1t_sb, in_=w1t[:].rearrange("(t p) o -> p t o", p=P)
            )
            w2t_sb = consts.tile([P, C], F32)
            nc.sync.dma_start(out=w2t_sb, in_=w2t[:])
            bns_sb = consts.tile([P, 1], F32)
            nc.sync.dma_start(out=bns_sb, in_=bns[:])
            bnb_sb = consts.tile([P, 1], F32)
            nc.sync.dma_start(out=bnb_sb, in_=bnb[:])

            # Pre-clear const dependencies (tiny dummy consumers).
            scratch_ps = psum.tile([P, 1], F32, tag="mlp", name="scratch_ps")
            nc.tensor.matmul(
                scratch_ps, ones128, ones128[:, :1], start=True, stop=True
            )
            nc.tensor.matmul(
                scratch_ps, w1t_sb[:, 0, :], ones128[:, :1], start=True, stop=True
            )
            nc.tensor.matmul(
                scratch_ps, w2t_sb[:, :P], ones128[:, :1], start=True, stop=True
            )
            scratch_sb = consts.tile([P, 1], F32)
            nc.scalar.copy(scratch_sb, bns_sb)
            nc.scalar.copy(scratch_sb, bnb_sb)

            # Quarter-granular loads, all up front on the SP HWDGE ring, in
            # (batch, quarter) order so the first compute quarter is ready
            # ~7us in. Stores go on the ACT ring.
            xq = [
                [[None] * QS for _ in range(CT)] for _ in range(BPC)
            ]
            for b in range(BPC):
                for q in range(QS):
                    for t in range(CT):
                        tile_ = xpool.tile(
                            [P, NQ], F32, tag="x", name=f"x_{b}_{t}_{q}"
                        )
                        nc.sync.dma_start(
                            out=tile_,
                            in_=x[b, t * P : (t + 1) * P, q * NQ : (q + 1) * NQ],
                        )
                        xq[b][t][q] = tile_

            attq_all = []
            hpsums = []
            for b in range(BPC):
                hpsums.append(
                    psum.tile([P, 1], F32, tag="mlp", name=f"hpsum_{b}")
                )
                attq_all.append(
                    [
                        [
                            small.tile(
                                [P, 1], F32, tag="attq", name=f"attq_{b}_{q}_{t}"
                            )
                            for t in range(CT)
                        ]
                        for q in range(QS)
                    ]
                )
            for b in range(BPC):
                attq = attq_all[b]
                for q in range(QS):
                    # tree-sum the 4 channel tiles (tA on DVE; tB + final sum
                    # on GpSimd, keeping DVE free for the fused att pass)
                    tA = big.tile([P, NQ], F32, tag="tA", bufs=2, name=f"tA_{b}_{q}")
                    tB = big.tile([P, NQ], F32, tag="tB", bufs=2, name=f"tB_{b}_{q}")
                    nc.vector.tensor_add(tA, xq[b][0][q], xq[b][1][q])
                    nc.gpsimd.tensor_add(tB, xq[b][2][q], xq[b][3][q])
                    nc.gpsimd.tensor_add(tB, tA, tB)
                    # sB[m, n] = colsum(tree) broadcast to all 128 output
                    # partitions in ONE matmul per chunk: ones[128,128]
                    # weights against the pre-reduced tree tile
                    sb = psum.tile(
                        [P, NQ], F32, tag="sb", bufs=3, name=f"sb_{b}_{q}"
                    )
                    for j in range(NQ // NCHUNK):
                        cols = slice(j * NCHUNK, (j + 1) * NCHUNK)
                        nc.tensor.matmul(
                            sb[:, cols], ones128, tB[:, cols],
                            start=True, stop=True,
                        )
                    for t in range(CT):
                        junk = big.tile(
                            [P, NQ], F32, tag="junk", bufs=1, name=f"junk_{b}_{q}_{t}"
                        )
                        # fused: junk = (x*1.0)*sb, attq = rowsum(junk)
                        nc.vector.scalar_tensor_tensor(
                            out=junk,
                            in0=xq[b][t][q],
                            scalar=1.0,
                            in1=sb,
                            op0=ALU.mult,
                            op1=ALU.mult,
                            accum_out=attq[q][t],
                        )
                        # fold W1T @ attq into the PSUM accumulation as the
                        # partials appear (removes the MLP from the tail)
                        nc.tensor.matmul(
                            hpsums[b],
                            w1t_sb[:, t, :],
                            attq[q][t],
                            start=(q == 0 and t == 0),
                            stop=(q == QS - 1 and t == CT - 1),
                        )

            for b in range(BPC):
                attq = attq_all[b]
                # h = relu(bn_scale * (W1 @ att) + bn_bias); the W1T @ att
                # accumulation already happened inside the quarter stream
                hb = small.tile([P, 1], F32, tag="hb", name=f"hb_{b}")
                nc.scalar.activation(
                    hb, hpsums[b], AF.Relu, bias=bnb_sb, scale=bns_sb
                )

                # a = sigmoid(W2 @ h), per 128-channel chunk
                avec = []
                for t in range(CT):
                    apsum = psum.tile(
                        [P, 1], F32, tag="mlp", name=f"apsum_{b}_{t}"
                    )
                    nc.tensor.matmul(
                        apsum,
                        w2t_sb[:, t * P : (t + 1) * P],
                        hb,
                        start=True,
                        stop=True,
                    )
                    a_t = small.tile([P, 1], F32, tag="a", name=f"a_{b}_{t}")
                    nc.scalar.activation(a_t, apsum, AF.Sigmoid)
                    avec.append(a_t)

                # out = x * a in place over each x quarter-tile (DVE/ACT
                # split), stored from it; last batch splits across both rings
                for t in range(CT):
                    for q in range(QS):
                        if (t + q) % 2 == 0:
                            nc.vector.tensor_scalar_mul(
                                xq[b][t][q], xq[b][t][q], avec[t]
                            )
                        else:
                            nc.scalar.mul(xq[b][t][q], xq[b][t][q], avec[t])
                        ring = nc.sync if t < 2 else nc.scalar
                        ring.dma_start(
                            out=y[b, t * P : (t + 1) * P, q * NQ : (q + 1) * NQ],
                            in_=xq[b][t][q],
                        )
    return nc


def _get_nc():
    global _nc_cache
    if _nc_cache is None:
        _nc_cache = _build()
        if not _nc_cache.is_finalized():
            _nc_cache.finalize()
    return _nc_cache


def _host_prep(x, W1, gamma, beta, running_mean, running_var, W2):
    x = np.asarray(x, dtype=np.float32)
    rstd = 1.0 / np.sqrt(np.asarray(running_var, np.float32) + BN_EPS)
    bns = (np.asarray(gamma, np.float32) * rstd).reshape(CR, 1)
    bnb = (
        np.asarray(beta, np.float32)
        - np.asarray(running_mean, np.float32) * bns[:, 0]
    ).reshape(CR, 1)
    w1t = np.ascontiguousarray(np.asarray(W1, np.float32).T)  # [C, CR]
    w2t = np.ascontiguousarray(np.asarray(W2, np.float32).T)  # [CR, C]
    in_maps = []
    for c in range(NCORES):
        in_maps.append(
            {
                "x": np.ascontiguousarray(x[c * BPC : (c + 1) * BPC]),
                "w1t": w1t,
                "w2t": w2t,
                "bns": np.ascontiguousarray(bns, np.float32),
                "bnb": np.ascontiguousarray(bnb, np.float32),
            }
        )
    return in_maps


def _run(inputs, **spmd_kwargs):
    in_maps = _host_prep(**inputs)
    res = run_bass_kernel_spmd(
        _get_nc(), in_maps, list(range(NCORES)), **spmd_kwargs
    )
    out = np.concatenate([res.results[c]["y"] for c in range(NCORES)], axis=0)
    return out.astype(np.float32, copy=False), res


def kernel(**inputs):
    out, _ = _run(inputs)
    return out



# revision 2
# speedup vs baseline: 1.0289x; 1.0289x over previous
"""Trainium2 Bass kernel for the SE-attention block.

Math (per batch b):
    s[n]   = sum_c x[b,c,n]
    att[c] = sum_n x[b,c,n] * s[n]
    h      = relu(bn(W1 @ att))          (BN folded into scale/bias on host)
    a      = sigmoid(W2 @ h)
    out    = x[b] * a[:, None]

Sharding: data-parallel over batch B=16 across 8 cores (2 batches/core),
weights replicated, no collectives. HBM traffic is fixed at ~33.5 MB per
core (16.8 in + 16.8 out) -> ~82us at the ~410 GB/s a single HWDGE queue
sustains; everything else must hide under it. Sigmoid inputs are huge so
the att path stays exact fp32 (bf16/fp32r flips near-threshold gates).

Schedule per core (two batches, each in four 1024-wide quarters):
  - SP HWDGE ring: all 32 quarter-loads up front, then ALL stores (the
    SP sequencer is idle after issuing loads, so it pays the ~590ns
    HWDGE issue cost for every store; ACT never stalls on store issue).
  - Const loads (W1T/W2T/BN) go on the ACT ring so they don't delay the
    first x quarters.
  - att pipeline: tA=x0+x1 on DVE (all four quarters up front so GpSimd
    is never starved), tB=x2+x3 and tA+tB on GpSimd, sB = ones^T @ tree
    broadcast matmul on PE (one matmul per 512-chunk, fp32), then
    attq = rowsum(x*sB) as fused scalar_tensor_tensor on DVE, with
    W1T@attq folded into a PSUM accumulation as partials appear.
  - muls: batch 0's 16 x*a tiles all on ACT (DVE must stay clear for
    batch 1's att work -- that chain gates the final stores); batch 1's
    split 8 DVE (747ns fast tensor_scalar) / 8 ACT.
"""

import numpy as np

try:
    import concourse.bass as bass
except ImportError:  # fresh grading dir: repo not on sys.path
    import sys

    for p in ("/opt/trn_rl_repo", "/root/.axon_site/_ro/trn_rl_repo"):
        if p not in sys.path:
            sys.path.insert(0, p)
    import concourse.bass as bass

import concourse.tile as tile
from concourse import bacc, mybir
from concourse.bass_utils import run_bass_kernel_spmd

F32 = mybir.dt.float32
AF = mybir.ActivationFunctionType
ALU = mybir.AluOpType

B, C, N = 16, 512, 4096
CR = 128          # squeeze dim C//4
NCORES = 8
BPC = B // NCORES  # batches per core
P = 128
CT = C // P        # channel tiles per batch
NCHUNK = 512       # matmul free-dim max (one psum bank)
BN_EPS = 1e-5

_nc_cache = None


def _build():
    nc = bacc.Bacc(None, target_bir_lowering=False)
    x = nc.declare_dram_parameter("x", [BPC, C, N], F32, isOutput=False)
    w1t = nc.declare_dram_parameter("w1t", [C, CR], F32, isOutput=False)
    w2t = nc.declare_dram_parameter("w2t", [CR, C], F32, isOutput=False)
    bns = nc.declare_dram_parameter("bns", [CR, 1], F32, isOutput=False)
    bnb = nc.declare_dram_parameter("bnb", [CR, 1], F32, isOutput=False)
    y = nc.declare_dram_parameter("y", [BPC, C, N], F32, isOutput=True)

    NQ = N // 4   # 1024-wide pipeline quarters
    QS = 4        # quarters per batch

    with tile.TileContext(nc) as tc:
        with (
            tc.tile_pool(name="consts", bufs=1) as consts,
            tc.tile_pool(name="x", bufs=2 * CT * QS) as xpool,
            tc.tile_pool(name="big", bufs=2) as big,
            tc.tile_pool(name="small", bufs=4 * CT) as small,
            tc.tile_pool(name="psum", bufs=2, space="PSUM") as psum,
            tc.tile_pool(name="dram", bufs=4, space="DRAM") as dram,
        ):
            ones128 = consts.tile([P, P], F32)
            nc.vector.memset(ones128, 1.0)
            # Const loads on the ACT HWDGE ring: x loads own the SP ring
            # from the first instruction.
            w1t_sb = consts.tile([P, CT, CR], F32)
            nc.scalar.dma_start(
                out=w1t_sb, in_=w1t[:].rearrange("(t p) o -> p t o", p=P)
            )
            w2t_sb = consts.tile([P, C], F32)
            nc.scalar.dma_start(out=w2t_sb, in_=w2t[:])
            bns_sb = consts.tile([P, 1], F32)
            nc.scalar.dma_start(out=bns_sb, in_=bns[:])
            bnb_sb = consts.tile([P, 1], F32)
            nc.scalar.dma_start(out=bnb_sb, in_=bnb[:])

            # Pre-clear const dependencies (tiny dummy consumers).
            scratch_ps = psum.tile([P, 1], F32, tag="mlp", name="scratch_ps")
            nc.tensor.matmul(
                scratch_ps, ones128, ones128[:, :1], start=True, stop=True
            )
            nc.tensor.matmul(
                scratch_ps, w1t_sb[:, 0, :], ones128[:, :1], start=True, stop=True
            )
            nc.tensor.matmul(
                scratch_ps, w2t_sb[:, :P], ones128[:, :1], start=True, stop=True
            )
            scratch_sb = consts.tile([P, 1], F32)
            nc.scalar.copy(scratch_sb, bns_sb)
            nc.scalar.copy(scratch_sb, bnb_sb)

            # Quarter-granular loads, all up front on the SP HWDGE ring, in
            # (batch, quarter) order so the first compute quarter is ready
            # ~7us in.
            xq = [
                [[None] * QS for _ in range(CT)] for _ in range(BPC)
            ]
            for b in range(BPC):
                for q in range(QS):
                    for t in range(CT):
                        tile_ = xpool.tile(
                            [P, NQ], F32, tag="x", name=f"x_{b}_{t}_{q}"
                        )
                        nc.sync.dma_start(
                            out=tile_,
                            in_=x[b, t * P : (t + 1) * P, q * NQ : (q + 1) * NQ],
                        )
                        xq[b][t][q] = tile_

            attq_all = []
            hpsums = []
            for b in range(BPC):
                hpsums.append(
                    psum.tile([P, 1], F32, tag="mlp", name=f"hpsum_{b}")
                )
                attq_all.append(
                    [
                        [
                            small.tile(
                                [P, 1], F32, tag="attq", name=f"attq_{b}_{q}_{t}"
                            )
                            for t in range(CT)
                        ]
                        for q in range(QS)
                    ]
                )
            for b in range(BPC):
                attq = attq_all[b]
                # All four tA adds up front on DVE: the GpSimd tree chain
                # (tB, tA+tB) is the slow serial producer feeding PE, so it
                # must never wait on a tA.
                tAs = []
                for q in range(QS):
                    tA = big.tile([P, NQ], F32, tag="tA", bufs=4, name=f"tA_{b}_{q}")
                    nc.vector.tensor_add(tA, xq[b][0][q], xq[b][1][q])
                    tAs.append(tA)
                for q in range(QS):
                    tB = big.tile([P, NQ], F32, tag="tB", bufs=2, name=f"tB_{b}_{q}")
                    nc.gpsimd.tensor_add(tB, xq[b][2][q], xq[b][3][q])
                    nc.gpsimd.tensor_add(tB, tAs[q], tB)
                    # sB[m, n] = colsum(tree) broadcast to all 128 output
                    # partitions in ONE matmul per chunk: ones[128,128]
                    # weights against the pre-reduced tree tile
                    sb = psum.tile(
                        [P, NQ], F32, tag="sb", bufs=3, name=f"sb_{b}_{q}"
                    )
                    for j in range(NQ // NCHUNK):
                        cols = slice(j * NCHUNK, (j + 1) * NCHUNK)
                        nc.tensor.matmul(
                            sb[:, cols], ones128, tB[:, cols],
                            start=True, stop=True,
                        )
                    for t in range(CT):
                        junk = big.tile(
                            [P, NQ], F32, tag="junk", bufs=1, name=f"junk_{b}_{q}_{t}"
                        )
                        # fused: junk = (x*1.0)*sb, attq = rowsum(junk)
                        nc.vector.scalar_tensor_tensor(
                            out=junk,
                            in0=xq[b][t][q],
                            scalar=1.0,
                            in1=sb,
                            op0=ALU.mult,
                            op1=ALU.mult,
                            accum_out=attq[q][t],
                        )
                        # fold W1T @ attq into the PSUM accumulation as the
                        # partials appear (removes the MLP from the tail)
                        nc.tensor.matmul(
                            hpsums[b],
                            w1t_sb[:, t, :],
                            attq[q][t],
                            start=(q == 0 and t == 0),
                            stop=(q == QS - 1 and t == CT - 1),
                        )

            for b in range(BPC):
                # h = relu(bn_scale * (W1 @ att) + bn_bias); the W1T @ att
                # accumulation already happened inside the quarter stream
                hb = small.tile([P, 1], F32, tag="hb", name=f"hb_{b}")
                nc.scalar.activation(
                    hb, hpsums[b], AF.Relu, bias=bnb_sb, scale=bns_sb
                )

                # a = sigmoid(W2 @ h), per 128-channel chunk
                avec = []
                for t in range(CT):
                    apsum = psum.tile(
                        [P, 1], F32, tag="mlp", name=f"apsum_{b}_{t}"
                    )
                    nc.tensor.matmul(
                        apsum,
                        w2t_sb[:, t * P : (t + 1) * P],
                        hb,
                        start=True,
                        stop=True,
                    )
                    a_t = small.tile([P, 1], F32, tag="a", name=f"a_{b}_{t}")
                    nc.scalar.activation(a_t, apsum, AF.Sigmoid)
                    avec.append(a_t)

                # out = x * a in place over each x quarter-tile; stores all
                # issue from the (idle) SP sequencer so ACT never pays DMA
                # issue cost. Non-final batches multiply entirely on ACT:
                # DVE must stay clear for the next batch's att chain.
                last = b == BPC - 1
                for t in range(CT):
                    for q in range(QS):
                        if last and (t + q) % 2 == 0:
                            nc.vector.tensor_scalar_mul(
                                xq[b][t][q], xq[b][t][q], avec[t]
                            )
                        else:
                            nc.scalar.mul(xq[b][t][q], xq[b][t][q], avec[t])
                        nc.sync.dma_start(
                            out=y[b, t * P : (t + 1) * P, q * NQ : (q + 1) * NQ],
                            in_=xq[b][t][q],
                        )
    return nc


def _get_nc():
    global _nc_cache
    if _nc_cache is None:
        _nc_cache = _build()
        if not _nc_cache.is_finalized():
            _nc_cache.finalize()
    return _nc_cache


def _host_prep(x, W1, gamma, beta, running_mean, running_var, W2):
    x = np.asarray(x, dtype=np.float32)
    rstd = 1.0 / np.sqrt(np.asarray(running_var, np.float32) + BN_EPS)
    bns = (np.asarray(gamma, np.float32) * rstd).reshape(CR, 1)
    bnb = (
        np.asarray(beta, np.float32)
        - np.asarray(running_mean, np.float32) * bns[:, 0]
    ).reshape(CR, 1)
    w1t = np.ascontiguousarray(np.asarray(W1, np.float32).T)  # [C, CR]
    w2t = np.ascontiguousarray(np.asarray(W2, np.float32).T)  # [CR, C]
    in_maps = []
    for c in range(NCORES):
        in_maps.append(
            {
                "x": np.ascontiguousarray(x[c * BPC : (c + 1) * BPC]),
                "w1t": w1t,
                "w2t": w2t,
                "bns": np.ascontiguousarray(bns, np.float32),
                "bnb": np.ascontiguousarray(bnb, np.float32),
            }
        )
    return in_maps


def _run(inputs, **spmd_kwargs):
    in_maps = _host_prep(**inputs)
    res = run_bass_kernel_spmd(
        _get_nc(), in_maps, list(range(NCORES)), **spmd_kwargs
    )
    out = np.concatenate([res.results[c]["y"] for c in range(NCORES)], axis=0)
    return out.astype(np.float32, copy=False), res


def kernel(**inputs):
    out, _ = _run(inputs)
    return out
